# revision 24
# baseline (speedup 1.0000x reference)
"""LSTM sequence classifier on 8 Trainium2 NeuronCores.

Data-parallel over batch: each core gets ~1/8 of the 4096 sequences.
Host pre-gathers token embeddings into a dense per-core stream (the
gather is pure data movement, done in numpy), so the device runs only
dense DMA + compute.  Per step the rhs operand packs [h; x; 1] into 5
K-tiles of 128 (h first so its partition layout matches the gate
layout; biases ride a constant-1 row), giving 12x5 matmuls per step.
All nonlinearities use sigmoid only (tanh x = 2*sigmoid(2x) - 1, with
the 2x folded into weights and the -0.5/x2 fixups folded into fused
DVE scalar_tensor_tensor ops; h is stored as h/2 with W_hh and the
head scale pre-doubled).  Gates live gate-major in PSUM: 4 banks per
step-segment, drained by a single sigmoid activation op.  Columns are
split into 2 interleaved groups x <=170-wide segments so ACT/DVE of
one unit overlap PE of the next and the recurrence never stalls PE.
"""
import sys

sys.path.insert(0, "/opt/trn_rl_repo")

import numpy as np
import ml_dtypes

import concourse.bass as bass
import concourse.tile as tile
from concourse import bacc, mybir
from concourse.bass_utils import run_bass_kernel_spmd

V, E, H, T, B = 30000, 300, 300, 22, 4096
NCORES = 8
KT = 5            # K-tiles: [h0, h1, h|x, x, x|1]
MW = 1536         # 4 gates x 384 padded rows
NMT = 12          # M-tiles
MAXW = 170        # max segment width (3*170 <= 512 psum bank f32)
F32 = mybir.dt.float32
BF16 = mybir.dt.bfloat16
FP16 = mybir.dt.float16
AF = mybir.ActivationFunctionType
ALU = mybir.AluOpType

_patched = False


def _patch_tile_drain():
    """walrus CTRL (Drain) supports fewer sem waits than Tile attaches at
    the kernel tail; spread them across single-wait SP NOPs instead."""
    global _patched
    if _patched:
        return
    _patched = True
    import concourse.tile as tile_mod
    from concourse.vector_clock import ScopedClock

    def _drain_and_barrier(self, tick_clock, wait_clock):
        nc = self.nc
        probe = nc.sync.nop(nofuse=True)
        wait_clock.add_sem_waits(
            probe.ins, ScopedClock({None: tick_clock.global_clock}))
        si = probe.ins.sync_info
        waits = list(si.on_wait) if si is not None else []
        upds = list(si.on_update) if si is not None else []
        probe.ins.sync_info = mybir.SyncInfo(on_wait=waits[:1], on_update=upds)
        for w in waits[1:]:
            n2 = nc.sync.nop(nofuse=True)
            n2.ins.sync_info = mybir.SyncInfo(on_wait=[w], on_update=[])
        nc.sync.drain()
        nc.all_engine_barrier()
        popped = nc._tile_sem_poison_stack.pop()
        assert popped is self._sem_poison
        nc.clear_and_free_semaphores(list(self.sems.allocated().values()))
        nc.all_engine_barrier()

    tile_mod.TileContext._drain_and_barrier = _drain_and_barrier


def _schedule(cap_len):
    """Deal batches to cores (identical length multiset per core), then
    deal each core's slots into 2 interleaved groups.

    Returns per-core per-group orders (global index or -1 for dummy) and
    per-group per-step active counts nA/nB (identical across cores).
    """
    orders = [([], []) for _ in range(NCORES)]
    qA = np.zeros(T + 1, np.int64)
    qB = np.zeros(T + 1, np.int64)
    toggle = 0
    for l in range(T, 0, -1):
        idxs = np.nonzero(cap_len == l)[0]
        ql = -(-len(idxs) // NCORES) if len(idxs) else 0
        parts = []
        for c in range(NCORES):
            p = [int(x) for x in idxs[c::NCORES]]
            parts.append(p + [-1] * (ql - len(p)))
        for j in range(ql):
            g = (toggle + j) % 2
            (qA if g == 0 else qB)[l] += 1
            for c in range(NCORES):
                orders[c][g].append(parts[c][j])
        toggle = (toggle + ql) % 2
    nA = [int(qA[t + 1:].sum()) for t in range(T)] + [0]
    nB = [int(qB[t + 1:].sum()) for t in range(T)] + [0]
    return orders, nA, nB


def _segments(n):
    """Split n active columns into balanced segments of width <= MAXW."""
    if n <= 0:
        return []
    S = -(-n // MAXW)
    w = -(-n // S)
    return [(s * w, min(n, (s + 1) * w)) for s in range(S)]


def _build_program(nG, offs, base, NTOKP, CQ, n0, dma_plan):
    nc = bacc.Bacc("TRN2", target_bir_lowering=False, debug=False)
    wxh_d = nc.dram_tensor("wxh", [KT, 128, MW], BF16, kind="ExternalInput")
    xab_d = nc.dram_tensor("xab", [128, 2, NTOKP], BF16, kind="ExternalInput")
    x2_d = nc.dram_tensor("x2", [84, NTOKP], BF16, kind="ExternalInput")
    xt0_d = nc.dram_tensor("xt0", [128, 3, n0], BF16, kind="ExternalInput")
    vt_d = nc.dram_tensor("vt", [128, 3, 2], BF16, kind="ExternalInput")
    s2_d = nc.dram_tensor("s2", [2, 1], F32, kind="ExternalInput")
    bc_d = nc.dram_tensor("bc", [2, 1], F32, kind="ExternalInput")
    out_d = nc.dram_tensor("out", [2, CQ], F32, kind="ExternalOutput")

    QA, QB = nG[0][0], nG[1][0]
    cbase = (0, QA)  # column base into cT/lastT/out_sb per group

    with tile.TileContext(nc) as tc:
        with (
            tc.tile_pool(name="const", bufs=1) as cpool,
            tc.tile_pool(name="gates", bufs=3) as gpool,
            tc.tile_pool(name="tsig", bufs=3) as tpool,
            tc.tile_pool(name="ps", bufs=4, space="PSUM") as pspool,
        ):
            wxh = cpool.tile([128, KT, MW], BF16, tag="wxh")
            xh = cpool.tile([128, KT, NTOKP], BF16, tag="xh")
            cT = cpool.tile([128, 3, CQ], FP16, tag="cT")
            lastT = cpool.tile([128, 3, CQ], BF16, tag="lastT")
            vt = cpool.tile([128, 3, 2], BF16, tag="vt")
            s2 = cpool.tile([2, 1], F32, tag="s2")
            bc = cpool.tile([2, 1], F32, tag="bc")
            out_sb = cpool.tile([2, CQ], F32, tag="out_sb")
            dum = cpool.tile([2, 2], F32, tag="dum")

            # Preload the sigmoid table while DMAs stream in.
            nc.vector.memset(dum[:], 0.0)
            nc.scalar.activation(dum[:], dum[:], AF.Sigmoid)

            # DMA issue plan: alternate between the two HWDGE queues.
            qeng = [nc.sync, nc.scalar]
            for qi, (kind, a) in enumerate(dma_plan):
                eng = qeng[qi % 2]
                if kind == "w":
                    eng.dma_start(out=wxh[:, a, :], in_=wxh_d[a])
                elif kind == "xt0":
                    d0, d1, s0, s1 = a
                    if d1 > d0:
                        eng.dma_start(out=xh[:, 2:5, d0:d1],
                                      in_=xt0_d[:, :, s0:s1])
                elif kind == "x2":
                    p0, p1 = a
                    if p1 > p0:
                        eng.dma_start(out=xh[44:128, 2, p0:p1],
                                      in_=x2_d[:, p0:p1])
                elif kind == "xab":
                    p0, p1 = a
                    if p1 > p0:
                        eng.dma_start(out=xh[:, 3:5, p0:p1],
                                      in_=xab_d[:, :, p0:p1])
                elif kind == "small":
                    eng.dma_start(out=vt[:], in_=vt_d[:])
                    eng.dma_start(out=s2[:], in_=s2_d[:])
                    eng.dma_start(out=bc[:], in_=bc_d[:])

            for t in range(T):
                units = []
                for g in (0, 1):
                    for si, seg in enumerate(_segments(nG[g][t])):
                        units.append((si, g, seg))
                units.sort()
                for (si, g, (s0, s1)) in units:
                    w = s1 - s0
                    P = base[g] + offs[g][t] + s0
                    # psum: bank-order [g,i,f,o].  start=True zeroes a whole
                    # 2KB bank (zero region): one start per bank on its first
                    # write, one stop on its last.  When 6w<=512 a unit fits
                    # TWO gates per bank -> one 2-bank tile, so 4 units can
                    # be in flight (deeper pipeline for the short-step tail).
                    single = 6 * w <= 512
                    if single:
                        pt = pspool.tile([128, 2, 512], F32, tag="ps")
                        ptiles = (pt, pt)

                        def oslice(b, sub, w=w, pt=pt):
                            c0 = (b % 2) * 3 * w + sub * w
                            return pt[:, b // 2, c0:c0 + w]
                        bfirst = (True, False, True, False)
                        blast = (False, True, False, True)
                    else:
                        pa = pspool.tile([128, 2, 512], F32, tag="ps")
                        pb = pspool.tile([128, 2, 512], F32, tag="ps")
                        ptiles = (pa, pb)

                        def oslice(b, sub, w=w, pa=pa, pb=pb):
                            tile_ = pa if b < 2 else pb
                            return tile_[:, b % 2, sub * w:(sub + 1) * w]
                        bfirst = (True, True, True, True)
                        blast = (True, True, True, True)
                    gb = gpool.tile([128, 4, 3 * MAXW], FP16, tag="gb")
                    tg = tpool.tile([128, 3 * MAXW], FP16, tag="tg")
                    klist = [3, 4, 2] if t == 0 else [3, 4, 2, 0, 1]
                    # phase 1: x-only K-tiles (no dependence on h)
                    for m in range(NMT):
                        b, sub = m // 3, m % 3
                        o = oslice(b, sub)
                        for k in klist[:2]:
                            nc.tensor.matmul(
                                o, wxh[:, k, m * 128:(m + 1) * 128],
                                xh[:, k, P:P + w],
                                start=(sub == 0 and k == klist[0]
                                       and bfirst[b]),
                                stop=False)
                    # phase 2: K-tiles that need h
                    for m in range(NMT):
                        b, sub = m // 3, m % 3
                        o = oslice(b, sub)
                        for k in klist[2:]:
                            nc.tensor.matmul(
                                o, wxh[:, k, m * 128:(m + 1) * 128],
                                xh[:, k, P:P + w],
                                start=False,
                                stop=(sub == 2 and k == klist[-1]
                                      and blast[b]))
                    # split drain: banks [g,i] first so the DVE tmp op can
                    # start while ACT drains [f,o] — shortens the chain and
                    # the ACT blocking quantum
                    if single:
                        d1_in = ptiles[0][:, 0, 0:6 * w]
                        d2_in = ptiles[1][:, 1, 0:6 * w]
                    else:
                        d1_in = ptiles[0][:, 0:2, 0:3 * w]
                        d2_in = ptiles[1][:, 0:2, 0:3 * w]
                    nc.scalar.activation(
                        gb[:, 0:2, 0:3 * w], d1_in, AF.Sigmoid)
                    csl = cT[:, :, cbase[g] + s0:cbase[g] + s1]
                    # tmp = (sig_g - 0.5)*sig_i = i*tanh(g)/2  -> gate-i slot
                    nc.vector.scalar_tensor_tensor(
                        gb[:, 1, 0:3 * w], gb[:, 0, 0:3 * w], -0.5,
                        gb[:, 1, 0:3 * w], op0=ALU.add, op1=ALU.mult)
                    nc.scalar.activation(
                        gb[:, 2:4, 0:3 * w], d2_in, AF.Sigmoid)
                    if t == 0:
                        nc.vector.tensor_scalar(
                            csl, gb[:, 1, 0:3 * w], 2.0, None, op0=ALU.mult)
                    else:
                        # f*c -> gate-f slot ; c = tmp*2 + f*c
                        nc.vector.scalar_tensor_tensor(
                            gb[:, 2, 0:3 * w], gb[:, 2, 0:3 * w], 0.0,
                            csl, op0=ALU.add, op1=ALU.mult)
                        nc.vector.scalar_tensor_tensor(
                            csl, gb[:, 1, 0:3 * w], 2.0,
                            gb[:, 2, 0:3 * w], op0=ALU.mult, op1=ALU.add)
                    # tg = sigmoid(2c);  h/2 = (tg - 0.5) * sig_o
                    nc.scalar.activation(
                        tg[:, 0:3 * w], csl, AF.Sigmoid, scale=2.0)
                    ncol = nG[g][t + 1]
                    se = min(s1, ncol)  # survivors in [s0, se)
                    if se > s0:
                        Pn = base[g] + offs[g][t + 1] + s0
                        wl = se - s0
                        # k2's h-residue first: phase 2 starts on it
                        nc.vector.scalar_tensor_tensor(
                            xh[0:44, 2, Pn:Pn + wl],
                            tg[0:44, 2 * w:2 * w + wl], -0.5,
                            gb[0:44, 3, 2 * w:2 * w + wl],
                            op0=ALU.add, op1=ALU.mult)
                        for sub in (0, 1):
                            nc.vector.scalar_tensor_tensor(
                                xh[:, sub, Pn:Pn + wl],
                                tg[:, sub * w:sub * w + wl], -0.5,
                                gb[:, 3, sub * w:sub * w + wl],
                                op0=ALU.add, op1=ALU.mult)
                    sd = max(s0, ncol)  # dying in [sd, s1)
                    if s1 > sd:
                        r0, r1 = sd - s0, s1 - s0
                        for sub in range(3):
                            nc.vector.scalar_tensor_tensor(
                                lastT[:, sub, cbase[g] + sd:cbase[g] + s1],
                                tg[:, sub * w + r0:sub * w + r1], -0.5,
                                gb[:, 3, sub * w + r0:sub * w + r1],
                                op0=ALU.add, op1=ALU.mult)

            # head: logits^T = s2 * (v @ last^T) + bc ; lastT holds h/2
            for g, Q in ((0, QA), (1, QB)):
                pht = pspool.tile([128, 2, 512], F32, tag="ps")
                ph = pht[0:2, 0, :]
                for k in range(3):
                    nc.tensor.matmul(ph[:, 0:Q], vt[:, k, :],
                                     lastT[:, k, cbase[g]:cbase[g] + Q],
                                     start=(k == 0), stop=(k == 2))
                nc.vector.tensor_scalar(
                    out_sb[:, cbase[g]:cbase[g] + Q], ph[:, 0:Q],
                    s2[:], bc[:], op0=ALU.mult, op1=ALU.add)
            nc.sync.dma_start(out=out_d[:], in_=out_sb[:])

    nc.compile()
    return nc


def _prepare(inputs):
    """Build the program and per-core input maps. Returns
    (nc, in_maps, meta) where meta has what output-unpacking needs."""
    _patch_tile_drain()
    cap = np.asarray(inputs["cap"]).astype(np.int64)
    cap_len = np.asarray(inputs["cap_len"]).astype(np.int64)
    embed = np.asarray(inputs["embed"], np.float32)
    W_ih = np.asarray(inputs["W_ih"], np.float32)
    W_hh = np.asarray(inputs["W_hh"], np.float32)
    b_ih = np.asarray(inputs["b_ih"], np.float32)
    b_hh = np.asarray(inputs["b_hh"], np.float32)
    v_wn = np.asarray(inputs["v_wn"], np.float32)
    g_wn = np.asarray(inputs["g_wn"], np.float32)
    b_cls = np.asarray(inputs["b_cls"], np.float32)

    orders, nA, nB = _schedule(cap_len)
    nGs = (nA, nB)
    offsA = np.concatenate([[0], np.cumsum(nA[:T])]).astype(np.int64)
    offsB = np.concatenate([[0], np.cumsum(nB[:T])]).astype(np.int64)
    NA, NB = int(offsA[T]), int(offsB[T])
    QA, QB = nA[0], nB[0]
    CQ = QA + QB
    NTOK = NA + NB
    NTOKP = NTOK + (-NTOK) % 16
    base = (0, NA)
    offs = (offsA, offsB)

    # ---- weights: contract rows [h(300)*2 ; x(300) ; 1-bias], M = 4x384
    # bank order g,i,f,o ; gate g rows are doubled for tanh-as-sigmoid.
    Wk = np.zeros((KT * 128, MW), np.float32)
    bias = b_ih + b_hh
    for b, gidx in enumerate((2, 0, 1, 3)):
        rows = slice(H * gidx, H * gidx + H)
        scale = 2.0 if gidx == 2 else 1.0
        Wk[0:H, 384 * b:384 * b + H] = 2.0 * scale * W_hh[rows, :].T
        Wk[300:600, 384 * b:384 * b + H] = scale * W_ih[rows, :].T
        Wk[600, 384 * b:384 * b + H] = scale * bias[rows]
    wxh_np = np.ascontiguousarray(
        Wk.reshape(KT, 128, MW)).astype(ml_dtypes.bfloat16)

    # head: s = 2 * g / ||v|| (factor 2 since lastT holds h/2)
    s2_np = (2.0 * g_wn / np.linalg.norm(v_wn, axis=1)).reshape(2, 1)
    s2_np = np.ascontiguousarray(s2_np, np.float32)
    bc_np = np.ascontiguousarray(b_cls.reshape(2, 1), np.float32)
    v_pad = np.zeros((384, 2), np.float32)
    v_pad[:H] = v_wn.T
    vt_np = np.ascontiguousarray(
        v_pad.reshape(3, 128, 2).transpose(1, 0, 2)).astype(
            ml_dtypes.bfloat16)

    emb_bf = embed.astype(ml_dtypes.bfloat16)

    # ---- per-core token streams and x layouts
    n0A, n0B = nA[0], nB[0]
    n0 = n0A + n0B
    in_maps = []
    for c in range(NCORES):
        toks = np.zeros(NTOKP, np.int64)
        for g in (0, 1):
            order = np.asarray(orders[c][g], np.int64)
            for t in range(T):
                n = nGs[g][t]
                if n == 0:
                    continue
                sel = order[:n]
                tk = np.where(sel >= 0, cap[np.clip(sel, 0, None), t], 0)
                toks[base[g] + offs[g][t]:base[g] + offs[g][t] + n] = tk
        X = emb_bf[toks]                      # [NTOKP, 300]
        XT = np.ascontiguousarray(X.T)        # [300, NTOKP]
        xab = np.zeros((128, 2, NTOKP), ml_dtypes.bfloat16)
        xab[:, 0, :] = XT[84:212]
        xab[0:88, 1, :] = XT[212:300]
        xab[88, 1, :] = 1.0
        x2 = np.ascontiguousarray(XT[0:84])   # -> xh[44:128, 2, :]
        # merged step-0 block: kt2 (zeros in h part) + kt3 + kt4
        xt0 = np.zeros((128, 3, n0), ml_dtypes.bfloat16)
        t0pos = np.concatenate(
            [np.arange(n0A), NA + np.arange(n0B)])
        xt0[44:128, 0, :] = XT[0:84][:, t0pos]
        xt0[:, 1, :] = XT[84:212][:, t0pos]
        xt0[0:88, 2, :] = XT[212:300][:, t0pos]
        xt0[88, 2, :] = 1.0
        in_maps.append({
            "wxh": wxh_np, "xab": xab, "x2": x2, "xt0": xt0,
            "vt": vt_np, "s2": s2_np, "bc": bc_np,
        })

    # ---- DMA issue plan (alternates between 2 queues in list order):
    # everything t=0 needs (both groups) first, then t=1, then the bulk.
    # xab/x2 cover [offs[1], N) only — xt0 carries the step-0 block.
    plan = [("w", 3), ("w", 4),
            ("xt0", (0, n0A, 0, n0A)),
            ("xt0", (NA, NA + n0B, n0A, n0)),
            ("w", 2), ("w", 0), ("w", 1)]
    cA = [int(offsA[t]) for t in (1, 2, 4, 8)] + [NA]
    cB = [NA + int(offsB[t]) for t in (1, 2, 4, 8)] + [NA + NB]
    plan += [("xab", (cA[0], cA[1])), ("xab", (cB[0], cB[1])),
             ("x2", (cA[0], cA[2])), ("x2", (cB[0], cB[2]))]
    for i in (1, 2, 3):
        plan += [("xab", (cA[i], cA[i + 1])), ("xab", (cB[i], cB[i + 1]))]
        if i >= 2:
            plan += [("x2", (cA[i], cA[i + 1])), ("x2", (cB[i], cB[i + 1]))]
    plan.append(("small", None))

    nc = _build_program(nGs, offs, base, NTOKP, CQ, n0, plan)
    return nc, in_maps, (orders, QA, QB)


def _unpack(results, meta):
    orders, QA, QB = meta
    out = np.zeros((B, 2), np.float32)
    for c in range(NCORES):
        logitsT = results[c]["out"]  # [2, CQ]
        for g, b0, Q in ((0, 0, QA), (1, QA, QB)):
            order = orders[c][g]
            for pos in range(Q):
                gi = order[pos]
                if gi >= 0:
                    out[gi] = logitsT[:, b0 + pos]
    return out


def _prep_and_run(inputs, trace=False):
    nc, in_maps, meta = _prepare(inputs)
    res = run_bass_kernel_spmd(nc, in_maps, list(range(NCORES)), trace=trace)
    return _unpack(res.results, meta), res


def kernel(**inputs):
    out, _ = _prep_and_run(inputs, trace=False)
    return out


# revision 26
# speedup vs baseline: 1.0600x; 1.0600x over previous
"""LSTM sequence classifier on 8 Trainium2 NeuronCores.

Data-parallel over batch: each core gets ~1/8 of the 4096 sequences.
Host pre-gathers token embeddings into a dense per-core stream (the
gather is pure data movement, done in numpy), so the device runs only
dense DMA + compute.  Per step the rhs operand packs [h; x; 1] into 5
K-tiles of 128 (h first so its partition layout matches the gate
layout; biases ride a constant-1 row), giving 12x5 matmuls per step.
All nonlinearities use sigmoid only (tanh x = 2*sigmoid(2x) - 1, with
the 2x folded into weights and the -0.5/x2 fixups folded into fused
DVE scalar_tensor_tensor ops; h is stored as h/2 with W_hh and the
head scale pre-doubled).  Gates live gate-major in PSUM: 4 banks per
step-segment, drained by a single sigmoid activation op.  Columns are
split into 2 interleaved groups x <=170-wide segments so ACT/DVE of
one unit overlap PE of the next and the recurrence never stalls PE.
"""
import sys

sys.path.insert(0, "/opt/trn_rl_repo")

import numpy as np
import ml_dtypes

import concourse.bass as bass
import concourse.tile as tile
from concourse import bacc, mybir
from concourse.bass_utils import run_bass_kernel_spmd

V, E, H, T, B = 30000, 300, 300, 22, 4096
NCORES = 8
KT = 5            # K-tiles: [h0, h1, h|x, x, x|1]
MW = 1536         # 4 gates x 384 padded rows
NMT = 12          # M-tiles
MAXW = 170        # max segment width (3*170 <= 512 psum bank f32)
F32 = mybir.dt.float32
BF16 = mybir.dt.bfloat16
FP16 = mybir.dt.float16
AF = mybir.ActivationFunctionType
ALU = mybir.AluOpType

_patched = False


def _patch_tile_drain():
    """walrus CTRL (Drain) supports fewer sem waits than Tile attaches at
    the kernel tail; spread them across single-wait SP NOPs instead."""
    global _patched
    if _patched:
        return
    _patched = True
    import concourse.tile as tile_mod
    from concourse.vector_clock import ScopedClock

    def _drain_and_barrier(self, tick_clock, wait_clock):
        nc = self.nc
        probe = nc.sync.nop(nofuse=True)
        wait_clock.add_sem_waits(
            probe.ins, ScopedClock({None: tick_clock.global_clock}))
        si = probe.ins.sync_info
        waits = list(si.on_wait) if si is not None else []
        upds = list(si.on_update) if si is not None else []
        probe.ins.sync_info = mybir.SyncInfo(on_wait=waits[:1], on_update=upds)
        for w in waits[1:]:
            n2 = nc.sync.nop(nofuse=True)
            n2.ins.sync_info = mybir.SyncInfo(on_wait=[w], on_update=[])
        nc.sync.drain()
        nc.all_engine_barrier()
        popped = nc._tile_sem_poison_stack.pop()
        assert popped is self._sem_poison
        nc.clear_and_free_semaphores(list(self.sems.allocated().values()))
        nc.all_engine_barrier()

    tile_mod.TileContext._drain_and_barrier = _drain_and_barrier


def _schedule(cap_len):
    """Deal batches to cores (identical length multiset per core), then
    deal each core's slots into 2 interleaved groups.

    Returns per-core per-group orders (global index or -1 for dummy) and
    per-group per-step active counts nA/nB (identical across cores).
    """
    orders = [([], []) for _ in range(NCORES)]
    qA = np.zeros(T + 1, np.int64)
    qB = np.zeros(T + 1, np.int64)
    toggle = 0
    for l in range(T, 0, -1):
        idxs = np.nonzero(cap_len == l)[0]
        ql = -(-len(idxs) // NCORES) if len(idxs) else 0
        parts = []
        for c in range(NCORES):
            p = [int(x) for x in idxs[c::NCORES]]
            parts.append(p + [-1] * (ql - len(p)))
        for j in range(ql):
            g = (toggle + j) % 2
            (qA if g == 0 else qB)[l] += 1
            for c in range(NCORES):
                orders[c][g].append(parts[c][j])
        toggle = (toggle + ql) % 2
    nA = [int(qA[t + 1:].sum()) for t in range(T)] + [0]
    nB = [int(qB[t + 1:].sum()) for t in range(T)] + [0]
    return orders, nA, nB


def _segments(n):
    """Split n active columns into balanced segments of width <= MAXW."""
    if n <= 0:
        return []
    S = -(-n // MAXW)
    w = -(-n // S)
    return [(s * w, min(n, (s + 1) * w)) for s in range(S)]


def _build_program(nG, offs, base, NTOKP, CQ, n0, dma_plan):
    nc = bacc.Bacc("TRN2", target_bir_lowering=False, debug=False)
    wxh_d = nc.dram_tensor("wxh", [KT, 128, MW], BF16, kind="ExternalInput")
    xab_d = nc.dram_tensor("xab", [128, 2, NTOKP], BF16, kind="ExternalInput")
    x2_d = nc.dram_tensor("x2", [84, NTOKP], BF16, kind="ExternalInput")
    xt0_d = nc.dram_tensor("xt0", [128, 3, n0], BF16, kind="ExternalInput")
    vt_d = nc.dram_tensor("vt", [128, 3, 2], BF16, kind="ExternalInput")
    s2_d = nc.dram_tensor("s2", [2, 1], F32, kind="ExternalInput")
    bc_d = nc.dram_tensor("bc", [2, 1], F32, kind="ExternalInput")
    out_d = nc.dram_tensor("out", [2, CQ], F32, kind="ExternalOutput")

    QA, QB = nG[0][0], nG[1][0]
    cbase = (0, QA)  # column base into cT/lastT/out_sb per group

    with tile.TileContext(nc) as tc:
        with (
            tc.tile_pool(name="const", bufs=1) as cpool,
            tc.tile_pool(name="gates", bufs=3) as gpool,
            tc.tile_pool(name="tsig", bufs=3) as tpool,
            tc.tile_pool(name="ps", bufs=4, space="PSUM") as pspool,
        ):
            wxh = cpool.tile([128, KT, MW], BF16, tag="wxh")
            xh = cpool.tile([128, KT, NTOKP], BF16, tag="xh")
            cT = cpool.tile([128, 3, CQ], FP16, tag="cT")
            lastT = cpool.tile([128, 3, CQ], BF16, tag="lastT")
            vt = cpool.tile([128, 3, 2], BF16, tag="vt")
            s2 = cpool.tile([2, 1], F32, tag="s2")
            bc = cpool.tile([2, 1], F32, tag="bc")
            out_sb = cpool.tile([2, CQ], F32, tag="out_sb")
            dum = cpool.tile([2, 2], F32, tag="dum")

            # DMA issue plan with explicit queues.  The scalar engine gets
            # ONLY the two t0-critical pieces: each dma_start occupies the
            # issuing engine until a HWDGE queue slot frees, so bulk DMAs
            # on the scalar engine would block the gate drains for ~20us.
            qeng = {"sy": nc.sync, "sc": nc.scalar}
            dummy_done = False
            for qi, (q, kind, a) in enumerate(dma_plan):
                eng = qeng[q]
                if q == "sy" and qi >= 2 and not dummy_done:
                    # preload the sigmoid table once the scalar engine has
                    # kicked its two critical DMAs
                    dummy_done = True
                    nc.vector.memset(dum[:], 0.0)
                    nc.scalar.activation(dum[:], dum[:], AF.Sigmoid)
                if kind == "w":
                    eng.dma_start(out=wxh[:, a, :], in_=wxh_d[a])
                elif kind == "xt0":
                    d0, d1, s0, s1 = a
                    if d1 > d0:
                        eng.dma_start(out=xh[:, 2:5, d0:d1],
                                      in_=xt0_d[:, :, s0:s1])
                elif kind == "x2":
                    p0, p1 = a
                    if p1 > p0:
                        eng.dma_start(out=xh[44:128, 2, p0:p1],
                                      in_=x2_d[:, p0:p1])
                elif kind == "xab":
                    p0, p1 = a
                    if p1 > p0:
                        eng.dma_start(out=xh[:, 3:5, p0:p1],
                                      in_=xab_d[:, :, p0:p1])
                elif kind == "small":
                    eng.dma_start(out=vt[:], in_=vt_d[:])
                    eng.dma_start(out=s2[:], in_=s2_d[:])
                    eng.dma_start(out=bc[:], in_=bc_d[:])

            for t in range(T):
                units = []
                for g in (0, 1):
                    for si, seg in enumerate(_segments(nG[g][t])):
                        units.append((si, g, seg))
                units.sort()
                for (si, g, (s0, s1)) in units:
                    w = s1 - s0
                    P = base[g] + offs[g][t] + s0
                    # psum: bank-order [g,i,f,o].  start=True zeroes a whole
                    # 2KB bank (zero region): one start per bank on its first
                    # write, one stop on its last.  When 6w<=512 a unit fits
                    # TWO gates per bank -> one 2-bank tile, so 4 units can
                    # be in flight (deeper pipeline for the short-step tail).
                    single = 6 * w <= 512
                    if single:
                        pt = pspool.tile([128, 2, 512], F32, tag="ps")
                        ptiles = (pt, pt)

                        def oslice(b, sub, w=w, pt=pt):
                            c0 = (b % 2) * 3 * w + sub * w
                            return pt[:, b // 2, c0:c0 + w]
                        bfirst = (True, False, True, False)
                        blast = (False, True, False, True)
                    else:
                        pa = pspool.tile([128, 2, 512], F32, tag="ps")
                        pb = pspool.tile([128, 2, 512], F32, tag="ps")
                        ptiles = (pa, pb)

                        def oslice(b, sub, w=w, pa=pa, pb=pb):
                            tile_ = pa if b < 2 else pb
                            return tile_[:, b % 2, sub * w:(sub + 1) * w]
                        bfirst = (True, True, True, True)
                        blast = (True, True, True, True)
                    gb = gpool.tile([128, 4, 3 * MAXW], FP16, tag="gb")
                    tg = tpool.tile([128, 3 * MAXW], FP16, tag="tg")
                    klist = [3, 4, 2] if t == 0 else [3, 4, 2, 0, 1]
                    # phase 1: x-only K-tiles (no dependence on h)
                    for m in range(NMT):
                        b, sub = m // 3, m % 3
                        o = oslice(b, sub)
                        for k in klist[:2]:
                            nc.tensor.matmul(
                                o, wxh[:, k, m * 128:(m + 1) * 128],
                                xh[:, k, P:P + w],
                                start=(sub == 0 and k == klist[0]
                                       and bfirst[b]),
                                stop=False)
                    # phase 2: K-tiles that need h
                    for m in range(NMT):
                        b, sub = m // 3, m % 3
                        o = oslice(b, sub)
                        for k in klist[2:]:
                            nc.tensor.matmul(
                                o, wxh[:, k, m * 128:(m + 1) * 128],
                                xh[:, k, P:P + w],
                                start=False,
                                stop=(sub == 2 and k == klist[-1]
                                      and blast[b]))
                    # split drain: banks [g,i] first so the DVE tmp op can
                    # start while ACT drains [f,o] — shortens the chain and
                    # the ACT blocking quantum
                    if single:
                        d1_in = ptiles[0][:, 0, 0:6 * w]
                        d2_in = ptiles[1][:, 1, 0:6 * w]
                    else:
                        d1_in = ptiles[0][:, 0:2, 0:3 * w]
                        d2_in = ptiles[1][:, 0:2, 0:3 * w]
                    nc.scalar.activation(
                        gb[:, 0:2, 0:3 * w], d1_in, AF.Sigmoid)
                    csl = cT[:, :, cbase[g] + s0:cbase[g] + s1]
                    # tmp = (sig_g - 0.5)*sig_i = i*tanh(g)/2  -> gate-i slot
                    nc.vector.scalar_tensor_tensor(
                        gb[:, 1, 0:3 * w], gb[:, 0, 0:3 * w], -0.5,
                        gb[:, 1, 0:3 * w], op0=ALU.add, op1=ALU.mult)
                    nc.scalar.activation(
                        gb[:, 2:4, 0:3 * w], d2_in, AF.Sigmoid)
                    if t == 0:
                        nc.vector.tensor_scalar(
                            csl, gb[:, 1, 0:3 * w], 2.0, None, op0=ALU.mult)
                    else:
                        # f*c -> gate-f slot ; c = tmp*2 + f*c
                        nc.vector.scalar_tensor_tensor(
                            gb[:, 2, 0:3 * w], gb[:, 2, 0:3 * w], 0.0,
                            csl, op0=ALU.add, op1=ALU.mult)
                        nc.vector.scalar_tensor_tensor(
                            csl, gb[:, 1, 0:3 * w], 2.0,
                            gb[:, 2, 0:3 * w], op0=ALU.mult, op1=ALU.add)
                    # tg = sigmoid(2c);  h/2 = (tg - 0.5) * sig_o
                    nc.scalar.activation(
                        tg[:, 0:3 * w], csl, AF.Sigmoid, scale=2.0)
                    ncol = nG[g][t + 1]
                    se = min(s1, ncol)  # survivors in [s0, se)
                    if se > s0:
                        Pn = base[g] + offs[g][t + 1] + s0
                        wl = se - s0
                        # k2's h-residue first: phase 2 starts on it
                        nc.vector.scalar_tensor_tensor(
                            xh[0:44, 2, Pn:Pn + wl],
                            tg[0:44, 2 * w:2 * w + wl], -0.5,
                            gb[0:44, 3, 2 * w:2 * w + wl],
                            op0=ALU.add, op1=ALU.mult)
                        for sub in (0, 1):
                            nc.vector.scalar_tensor_tensor(
                                xh[:, sub, Pn:Pn + wl],
                                tg[:, sub * w:sub * w + wl], -0.5,
                                gb[:, 3, sub * w:sub * w + wl],
                                op0=ALU.add, op1=ALU.mult)
                    sd = max(s0, ncol)  # dying in [sd, s1)
                    if s1 > sd:
                        r0, r1 = sd - s0, s1 - s0
                        for sub in range(3):
                            nc.vector.scalar_tensor_tensor(
                                lastT[:, sub, cbase[g] + sd:cbase[g] + s1],
                                tg[:, sub * w + r0:sub * w + r1], -0.5,
                                gb[:, 3, sub * w + r0:sub * w + r1],
                                op0=ALU.add, op1=ALU.mult)

            # head: logits^T = s2 * (v @ last^T) + bc ; lastT holds h/2
            for g, Q in ((0, QA), (1, QB)):
                pht = pspool.tile([128, 2, 512], F32, tag="ps")
                ph = pht[0:2, 0, :]
                for k in range(3):
                    nc.tensor.matmul(ph[:, 0:Q], vt[:, k, :],
                                     lastT[:, k, cbase[g]:cbase[g] + Q],
                                     start=(k == 0), stop=(k == 2))
                nc.vector.tensor_scalar(
                    out_sb[:, cbase[g]:cbase[g] + Q], ph[:, 0:Q],
                    s2[:], bc[:], op0=ALU.mult, op1=ALU.add)
            nc.sync.dma_start(out=out_d[:], in_=out_sb[:])

    nc.compile()
    return nc


def _prepare(inputs):
    """Build the program and per-core input maps. Returns
    (nc, in_maps, meta) where meta has what output-unpacking needs."""
    _patch_tile_drain()
    cap = np.asarray(inputs["cap"]).astype(np.int64)
    cap_len = np.asarray(inputs["cap_len"]).astype(np.int64)
    embed = np.asarray(inputs["embed"], np.float32)
    W_ih = np.asarray(inputs["W_ih"], np.float32)
    W_hh = np.asarray(inputs["W_hh"], np.float32)
    b_ih = np.asarray(inputs["b_ih"], np.float32)
    b_hh = np.asarray(inputs["b_hh"], np.float32)
    v_wn = np.asarray(inputs["v_wn"], np.float32)
    g_wn = np.asarray(inputs["g_wn"], np.float32)
    b_cls = np.asarray(inputs["b_cls"], np.float32)

    orders, nA, nB = _schedule(cap_len)
    nGs = (nA, nB)
    offsA = np.concatenate([[0], np.cumsum(nA[:T])]).astype(np.int64)
    offsB = np.concatenate([[0], np.cumsum(nB[:T])]).astype(np.int64)
    NA, NB = int(offsA[T]), int(offsB[T])
    QA, QB = nA[0], nB[0]
    CQ = QA + QB
    NTOK = NA + NB
    NTOKP = NTOK + (-NTOK) % 16
    base = (0, NA)
    offs = (offsA, offsB)

    # ---- weights: contract rows [h(300)*2 ; x(300) ; 1-bias], M = 4x384
    # bank order g,i,f,o ; gate g rows are doubled for tanh-as-sigmoid.
    Wk = np.zeros((KT * 128, MW), np.float32)
    bias = b_ih + b_hh
    for b, gidx in enumerate((2, 0, 1, 3)):
        rows = slice(H * gidx, H * gidx + H)
        scale = 2.0 if gidx == 2 else 1.0
        Wk[0:H, 384 * b:384 * b + H] = 2.0 * scale * W_hh[rows, :].T
        Wk[300:600, 384 * b:384 * b + H] = scale * W_ih[rows, :].T
        Wk[600, 384 * b:384 * b + H] = scale * bias[rows]
    wxh_np = np.ascontiguousarray(
        Wk.reshape(KT, 128, MW)).astype(ml_dtypes.bfloat16)

    # head: s = 2 * g / ||v|| (factor 2 since lastT holds h/2)
    s2_np = (2.0 * g_wn / np.linalg.norm(v_wn, axis=1)).reshape(2, 1)
    s2_np = np.ascontiguousarray(s2_np, np.float32)
    bc_np = np.ascontiguousarray(b_cls.reshape(2, 1), np.float32)
    v_pad = np.zeros((384, 2), np.float32)
    v_pad[:H] = v_wn.T
    vt_np = np.ascontiguousarray(
        v_pad.reshape(3, 128, 2).transpose(1, 0, 2)).astype(
            ml_dtypes.bfloat16)

    emb_bf = embed.astype(ml_dtypes.bfloat16)

    # ---- per-core token streams and x layouts
    n0A, n0B = nA[0], nB[0]
    n0 = n0A + n0B
    in_maps = []
    for c in range(NCORES):
        toks = np.zeros(NTOKP, np.int64)
        for g in (0, 1):
            order = np.asarray(orders[c][g], np.int64)
            for t in range(T):
                n = nGs[g][t]
                if n == 0:
                    continue
                sel = order[:n]
                tk = np.where(sel >= 0, cap[np.clip(sel, 0, None), t], 0)
                toks[base[g] + offs[g][t]:base[g] + offs[g][t] + n] = tk
        X = emb_bf[toks]                      # [NTOKP, 300]
        XT = np.ascontiguousarray(X.T)        # [300, NTOKP]
        xab = np.zeros((128, 2, NTOKP), ml_dtypes.bfloat16)
        xab[:, 0, :] = XT[84:212]
        xab[0:88, 1, :] = XT[212:300]
        xab[88, 1, :] = 1.0
        x2 = np.ascontiguousarray(XT[0:84])   # -> xh[44:128, 2, :]
        # merged step-0 block: kt2 (zeros in h part) + kt3 + kt4
        xt0 = np.zeros((128, 3, n0), ml_dtypes.bfloat16)
        t0pos = np.concatenate(
            [np.arange(n0A), NA + np.arange(n0B)])
        xt0[44:128, 0, :] = XT[0:84][:, t0pos]
        xt0[:, 1, :] = XT[84:212][:, t0pos]
        xt0[0:88, 2, :] = XT[212:300][:, t0pos]
        xt0[88, 2, :] = 1.0
        in_maps.append({
            "wxh": wxh_np, "xab": xab, "x2": x2, "xt0": xt0,
            "vt": vt_np, "s2": s2_np, "bc": bc_np,
        })

    # ---- DMA issue plan: scalar engine gets only w4+xt0B (t0-critical,
    # issued before its first activation); everything else streams on the
    # sync queue in need-order.  xab/x2 cover [offs[1], N) only — xt0
    # carries the step-0 block.
    plan = [("sc", "w", 4),
            ("sc", "xt0", (NA, NA + n0B, n0A, n0)),
            ("sy", "w", 3),
            ("sy", "xt0", (0, n0A, 0, n0A)),
            ("sy", "w", 2), ("sy", "w", 0), ("sy", "w", 1)]
    cA = [int(offsA[t]) for t in (1, 2, 4, 8)] + [NA]
    cB = [NA + int(offsB[t]) for t in (1, 2, 4, 8)] + [NA + NB]
    plan += [("sy", "xab", (cA[0], cA[1])), ("sy", "xab", (cB[0], cB[1])),
             ("sy", "x2", (cA[0], cA[2])), ("sy", "x2", (cB[0], cB[2]))]
    for i in (1, 2, 3):
        plan += [("sy", "xab", (cA[i], cA[i + 1])),
                 ("sy", "xab", (cB[i], cB[i + 1]))]
        if i >= 2:
            plan += [("sy", "x2", (cA[i], cA[i + 1])),
                     ("sy", "x2", (cB[i], cB[i + 1]))]
    plan.append(("sy", "small", None))

    nc = _build_program(nGs, offs, base, NTOKP, CQ, n0, plan)
    return nc, in_maps, (orders, QA, QB)


def _unpack(results, meta):
    orders, QA, QB = meta
    out = np.zeros((B, 2), np.float32)
    for c in range(NCORES):
        logitsT = results[c]["out"]  # [2, CQ]
        for g, b0, Q in ((0, 0, QA), (1, QA, QB)):
            order = orders[c][g]
            for pos in range(Q):
                gi = order[pos]
                if gi >= 0:
                    out[gi] = logitsT[:, b0 + pos]
    return out


def _prep_and_run(inputs, trace=False):
    nc, in_maps, meta = _prepare(inputs)
    res = run_bass_kernel_spmd(nc, in_maps, list(range(NCORES)), trace=trace)
    return _unpack(res.results, meta), res


def kernel(**inputs):
    out, _ = _prep_and_run(inputs, trace=False)
    return out


# revision 28
# speedup vs baseline: 1.0697x; 1.0091x over previous
"""LSTM sequence classifier on 8 Trainium2 NeuronCores.

Data-parallel over batch: each core gets ~1/8 of the 4096 sequences.
Host pre-gathers token embeddings into a dense per-core stream (the
gather is pure data movement, done in numpy), so the device runs only
dense DMA + compute.  Per step the rhs operand packs [h; x; 1] into 5
K-tiles of 128 (h first so its partition layout matches the gate
layout; biases ride a constant-1 row), giving 12x5 matmuls per step.
All nonlinearities use sigmoid only (tanh x = 2*sigmoid(2x) - 1, with
the 2x folded into weights and the -0.5/x2 fixups folded into fused
DVE scalar_tensor_tensor ops; h is stored as h/2 with W_hh and the
head scale pre-doubled).  Gates live gate-major in PSUM: 4 banks per
step-segment, drained by a single sigmoid activation op.  Columns are
split into 2 interleaved groups x <=170-wide segments so ACT/DVE of
one unit overlap PE of the next and the recurrence never stalls PE.
"""
import sys

sys.path.insert(0, "/opt/trn_rl_repo")

import numpy as np
import ml_dtypes

import concourse.bass as bass
import concourse.tile as tile
from concourse import bacc, mybir
from concourse.bass_utils import run_bass_kernel_spmd

V, E, H, T, B = 30000, 300, 300, 22, 4096
NCORES = 8
KT = 5            # K-tiles: [h0, h1, h|x, x, x|1]
MW = 1536         # 4 gates x 384 padded rows
NMT = 12          # M-tiles
MAXW = 170        # max segment width (3*170 <= 512 psum bank f32)
F32 = mybir.dt.float32
BF16 = mybir.dt.bfloat16
FP16 = mybir.dt.float16
AF = mybir.ActivationFunctionType
ALU = mybir.AluOpType

_patched = False


def _patch_tile_drain():
    """walrus CTRL (Drain) supports fewer sem waits than Tile attaches at
    the kernel tail; spread them across single-wait SP NOPs instead."""
    global _patched
    if _patched:
        return
    _patched = True
    import concourse.tile as tile_mod
    from concourse.vector_clock import ScopedClock

    def _drain_and_barrier(self, tick_clock, wait_clock):
        nc = self.nc
        probe = nc.sync.nop(nofuse=True)
        wait_clock.add_sem_waits(
            probe.ins, ScopedClock({None: tick_clock.global_clock}))
        si = probe.ins.sync_info
        waits = list(si.on_wait) if si is not None else []
        upds = list(si.on_update) if si is not None else []
        probe.ins.sync_info = mybir.SyncInfo(on_wait=waits[:1], on_update=upds)
        for w in waits[1:]:
            n2 = nc.sync.nop(nofuse=True)
            n2.ins.sync_info = mybir.SyncInfo(on_wait=[w], on_update=[])
        nc.sync.drain()
        nc.all_engine_barrier()
        popped = nc._tile_sem_poison_stack.pop()
        assert popped is self._sem_poison
        nc.clear_and_free_semaphores(list(self.sems.allocated().values()))
        nc.all_engine_barrier()

    tile_mod.TileContext._drain_and_barrier = _drain_and_barrier


def _schedule(cap_len):
    """Deal batches to cores (identical length multiset per core), then
    deal each core's slots into 2 interleaved groups.

    Returns per-core per-group orders (global index or -1 for dummy) and
    per-group per-step active counts nA/nB (identical across cores).
    """
    orders = [([], []) for _ in range(NCORES)]
    qA = np.zeros(T + 1, np.int64)
    qB = np.zeros(T + 1, np.int64)
    toggle = 0
    for l in range(T, 0, -1):
        idxs = np.nonzero(cap_len == l)[0]
        ql = -(-len(idxs) // NCORES) if len(idxs) else 0
        parts = []
        for c in range(NCORES):
            p = [int(x) for x in idxs[c::NCORES]]
            parts.append(p + [-1] * (ql - len(p)))
        for j in range(ql):
            g = (toggle + j) % 2
            (qA if g == 0 else qB)[l] += 1
            for c in range(NCORES):
                orders[c][g].append(parts[c][j])
        toggle = (toggle + ql) % 2
    nA = [int(qA[t + 1:].sum()) for t in range(T)] + [0]
    nB = [int(qB[t + 1:].sum()) for t in range(T)] + [0]
    return orders, nA, nB


def _segments(n):
    """Split n active columns into balanced segments of width <= MAXW."""
    if n <= 0:
        return []
    S = -(-n // MAXW)
    w = -(-n // S)
    return [(s * w, min(n, (s + 1) * w)) for s in range(S)]


def _build_program(nG, offs, base, NTOKP, CQ, n0, dma_plan):
    nc = bacc.Bacc("TRN2", target_bir_lowering=False, debug=False)
    wxh_d = nc.dram_tensor("wxh", [KT, 128, MW], BF16, kind="ExternalInput")
    xab_d = nc.dram_tensor("xab", [128, 2, NTOKP], BF16, kind="ExternalInput")
    x2_d = nc.dram_tensor("x2", [84, NTOKP], BF16, kind="ExternalInput")
    xt0_d = nc.dram_tensor("xt0", [128, 3, n0], BF16, kind="ExternalInput")
    vt_d = nc.dram_tensor("vt", [128, 3, 2], BF16, kind="ExternalInput")
    s2_d = nc.dram_tensor("s2", [2, 1], F32, kind="ExternalInput")
    bc_d = nc.dram_tensor("bc", [2, 1], F32, kind="ExternalInput")
    out_d = nc.dram_tensor("out", [2, CQ], F32, kind="ExternalOutput")

    QA, QB = nG[0][0], nG[1][0]
    cbase = (0, QA)  # column base into cT/lastT/out_sb per group

    with tile.TileContext(nc) as tc:
        with (
            tc.tile_pool(name="const", bufs=1) as cpool,
            tc.tile_pool(name="gates", bufs=5) as gpool,
            tc.tile_pool(name="tsig", bufs=5) as tpool,
            tc.tile_pool(name="ps", bufs=4, space="PSUM") as pspool,
        ):
            wxh = cpool.tile([128, KT, MW], BF16, tag="wxh")
            xh = cpool.tile([128, KT, NTOKP], BF16, tag="xh")
            cT = cpool.tile([128, 3, CQ], FP16, tag="cT")
            lastT = cpool.tile([128, 3, CQ], BF16, tag="lastT")
            vt = cpool.tile([128, 3, 2], BF16, tag="vt")
            s2 = cpool.tile([2, 1], F32, tag="s2")
            bc = cpool.tile([2, 1], F32, tag="bc")
            out_sb = cpool.tile([2, CQ], F32, tag="out_sb")
            dum = cpool.tile([2, 2], F32, tag="dum")

            # DMA issue plan with explicit queues.  The scalar engine gets
            # ONLY the two t0-critical pieces: each dma_start occupies the
            # issuing engine until a HWDGE queue slot frees, so bulk DMAs
            # on the scalar engine would block the gate drains for ~20us.
            qeng = {"sy": nc.sync, "sc": nc.scalar}
            dummy_done = False
            for qi, (q, kind, a) in enumerate(dma_plan):
                eng = qeng[q]
                if q == "sy" and qi >= 2 and not dummy_done:
                    # preload the sigmoid table once the scalar engine has
                    # kicked its two critical DMAs
                    dummy_done = True
                    nc.vector.memset(dum[:], 0.0)
                    nc.scalar.activation(dum[:], dum[:], AF.Sigmoid)
                if kind == "w":
                    eng.dma_start(out=wxh[:, a, :], in_=wxh_d[a])
                elif kind == "xt0":
                    d0, d1, s0, s1 = a
                    if d1 > d0:
                        eng.dma_start(out=xh[:, 2:5, d0:d1],
                                      in_=xt0_d[:, :, s0:s1])
                elif kind == "x2":
                    p0, p1 = a
                    if p1 > p0:
                        eng.dma_start(out=xh[44:128, 2, p0:p1],
                                      in_=x2_d[:, p0:p1])
                elif kind == "xab":
                    p0, p1 = a
                    if p1 > p0:
                        eng.dma_start(out=xh[:, 3:5, p0:p1],
                                      in_=xab_d[:, :, p0:p1])
                elif kind == "small":
                    eng.dma_start(out=vt[:], in_=vt_d[:])
                    eng.dma_start(out=s2[:], in_=s2_d[:])
                    eng.dma_start(out=bc[:], in_=bc_d[:])

            for t in range(T):
                units = []
                for g in (0, 1):
                    for si, seg in enumerate(_segments(nG[g][t])):
                        units.append((si, g, seg))
                units.sort()
                for (si, g, (s0, s1)) in units:
                    w = s1 - s0
                    P = base[g] + offs[g][t] + s0
                    # psum: bank-order [g,i,f,o].  start=True zeroes a whole
                    # 2KB bank (zero region): one start per bank on its first
                    # write, one stop on its last.  When 6w<=512 a unit fits
                    # TWO gates per bank -> one 2-bank tile, so 4 units can
                    # be in flight (deeper pipeline for the short-step tail).
                    single = 6 * w <= 512
                    if single:
                        pt = pspool.tile([128, 2, 512], F32, tag="ps")
                        ptiles = (pt, pt)

                        def oslice(b, sub, w=w, pt=pt):
                            c0 = (b % 2) * 3 * w + sub * w
                            return pt[:, b // 2, c0:c0 + w]
                        bfirst = (True, False, True, False)
                        blast = (False, True, False, True)
                    else:
                        pa = pspool.tile([128, 2, 512], F32, tag="ps")
                        pb = pspool.tile([128, 2, 512], F32, tag="ps")
                        ptiles = (pa, pb)

                        def oslice(b, sub, w=w, pa=pa, pb=pb):
                            tile_ = pa if b < 2 else pb
                            return tile_[:, b % 2, sub * w:(sub + 1) * w]
                        bfirst = (True, True, True, True)
                        blast = (True, True, True, True)
                    gb = gpool.tile([128, 4, 3 * MAXW], FP16, tag="gb")
                    tg = tpool.tile([128, 3 * MAXW], FP16, tag="tg")
                    klist = [3, 4, 2] if t == 0 else [3, 4, 2, 0, 1]
                    # phase 1: x-only K-tiles (no dependence on h).
                    # k-major order: phase 2's k2 needs only the h-residue
                    # DVE write, so later h writes get time to land.
                    for k in klist[:2]:
                        for m in range(NMT):
                            b, sub = m // 3, m % 3
                            nc.tensor.matmul(
                                oslice(b, sub),
                                wxh[:, k, m * 128:(m + 1) * 128],
                                xh[:, k, P:P + w],
                                start=(sub == 0 and k == klist[0]
                                       and bfirst[b]),
                                stop=False)
                    # phase 2: K-tiles that need h
                    for k in klist[2:]:
                        for m in range(NMT):
                            b, sub = m // 3, m % 3
                            nc.tensor.matmul(
                                oslice(b, sub),
                                wxh[:, k, m * 128:(m + 1) * 128],
                                xh[:, k, P:P + w],
                                start=False,
                                stop=(sub == 2 and k == klist[-1]
                                      and blast[b]))
                    # split drain: banks [g,i] first so the DVE tmp op can
                    # start while ACT drains [f,o] — shortens the chain and
                    # the ACT blocking quantum
                    if single:
                        d1_in = ptiles[0][:, 0, 0:6 * w]
                        d2_in = ptiles[1][:, 1, 0:6 * w]
                    else:
                        d1_in = ptiles[0][:, 0:2, 0:3 * w]
                        d2_in = ptiles[1][:, 0:2, 0:3 * w]
                    nc.scalar.activation(
                        gb[:, 0:2, 0:3 * w], d1_in, AF.Sigmoid)
                    csl = cT[:, :, cbase[g] + s0:cbase[g] + s1]
                    # tmp = (sig_g - 0.5)*sig_i = i*tanh(g)/2  -> gate-i slot
                    nc.vector.scalar_tensor_tensor(
                        gb[:, 1, 0:3 * w], gb[:, 0, 0:3 * w], -0.5,
                        gb[:, 1, 0:3 * w], op0=ALU.add, op1=ALU.mult)
                    nc.scalar.activation(
                        gb[:, 2:4, 0:3 * w], d2_in, AF.Sigmoid)
                    if t == 0:
                        nc.vector.tensor_scalar(
                            csl, gb[:, 1, 0:3 * w], 2.0, None, op0=ALU.mult)
                    else:
                        # f*c -> gate-f slot ; c = tmp*2 + f*c
                        nc.vector.scalar_tensor_tensor(
                            gb[:, 2, 0:3 * w], gb[:, 2, 0:3 * w], 0.0,
                            csl, op0=ALU.add, op1=ALU.mult)
                        nc.vector.scalar_tensor_tensor(
                            csl, gb[:, 1, 0:3 * w], 2.0,
                            gb[:, 2, 0:3 * w], op0=ALU.mult, op1=ALU.add)
                    # tg = sigmoid(2c);  h/2 = (tg - 0.5) * sig_o
                    nc.scalar.activation(
                        tg[:, 0:3 * w], csl, AF.Sigmoid, scale=2.0)
                    ncol = nG[g][t + 1]
                    se = min(s1, ncol)  # survivors in [s0, se)
                    if se > s0:
                        Pn = base[g] + offs[g][t + 1] + s0
                        wl = se - s0
                        # k2's h-residue first: phase 2 starts on it
                        nc.vector.scalar_tensor_tensor(
                            xh[0:44, 2, Pn:Pn + wl],
                            tg[0:44, 2 * w:2 * w + wl], -0.5,
                            gb[0:44, 3, 2 * w:2 * w + wl],
                            op0=ALU.add, op1=ALU.mult)
                        for sub in (0, 1):
                            nc.vector.scalar_tensor_tensor(
                                xh[:, sub, Pn:Pn + wl],
                                tg[:, sub * w:sub * w + wl], -0.5,
                                gb[:, 3, sub * w:sub * w + wl],
                                op0=ALU.add, op1=ALU.mult)
                    sd = max(s0, ncol)  # dying in [sd, s1)
                    if s1 > sd:
                        r0, r1 = sd - s0, s1 - s0
                        for sub in range(3):
                            nc.vector.scalar_tensor_tensor(
                                lastT[:, sub, cbase[g] + sd:cbase[g] + s1],
                                tg[:, sub * w + r0:sub * w + r1], -0.5,
                                gb[:, 3, sub * w + r0:sub * w + r1],
                                op0=ALU.add, op1=ALU.mult)

            # head: logits^T = s2 * (v @ last^T) + bc ; lastT holds h/2
            for g, Q in ((0, QA), (1, QB)):
                pht = pspool.tile([128, 2, 512], F32, tag="ps")
                ph = pht[0:2, 0, :]
                for k in range(3):
                    nc.tensor.matmul(ph[:, 0:Q], vt[:, k, :],
                                     lastT[:, k, cbase[g]:cbase[g] + Q],
                                     start=(k == 0), stop=(k == 2))
                nc.vector.tensor_scalar(
                    out_sb[:, cbase[g]:cbase[g] + Q], ph[:, 0:Q],
                    s2[:], bc[:], op0=ALU.mult, op1=ALU.add)
            nc.sync.dma_start(out=out_d[:], in_=out_sb[:])

    nc.compile()
    return nc


def _prepare(inputs):
    """Build the program and per-core input maps. Returns
    (nc, in_maps, meta) where meta has what output-unpacking needs."""
    _patch_tile_drain()
    cap = np.asarray(inputs["cap"]).astype(np.int64)
    cap_len = np.asarray(inputs["cap_len"]).astype(np.int64)
    embed = np.asarray(inputs["embed"], np.float32)
    W_ih = np.asarray(inputs["W_ih"], np.float32)
    W_hh = np.asarray(inputs["W_hh"], np.float32)
    b_ih = np.asarray(inputs["b_ih"], np.float32)
    b_hh = np.asarray(inputs["b_hh"], np.float32)
    v_wn = np.asarray(inputs["v_wn"], np.float32)
    g_wn = np.asarray(inputs["g_wn"], np.float32)
    b_cls = np.asarray(inputs["b_cls"], np.float32)

    orders, nA, nB = _schedule(cap_len)
    nGs = (nA, nB)
    offsA = np.concatenate([[0], np.cumsum(nA[:T])]).astype(np.int64)
    offsB = np.concatenate([[0], np.cumsum(nB[:T])]).astype(np.int64)
    NA, NB = int(offsA[T]), int(offsB[T])
    QA, QB = nA[0], nB[0]
    CQ = QA + QB
    NTOK = NA + NB
    NTOKP = NTOK + (-NTOK) % 16
    base = (0, NA)
    offs = (offsA, offsB)

    # ---- weights: contract rows [h(300)*2 ; x(300) ; 1-bias], M = 4x384
    # bank order g,i,f,o ; gate g rows are doubled for tanh-as-sigmoid.
    Wk = np.zeros((KT * 128, MW), np.float32)
    bias = b_ih + b_hh
    for b, gidx in enumerate((2, 0, 1, 3)):
        rows = slice(H * gidx, H * gidx + H)
        scale = 2.0 if gidx == 2 else 1.0
        Wk[0:H, 384 * b:384 * b + H] = 2.0 * scale * W_hh[rows, :].T
        Wk[300:600, 384 * b:384 * b + H] = scale * W_ih[rows, :].T
        Wk[600, 384 * b:384 * b + H] = scale * bias[rows]
    wxh_np = np.ascontiguousarray(
        Wk.reshape(KT, 128, MW)).astype(ml_dtypes.bfloat16)

    # head: s = 2 * g / ||v|| (factor 2 since lastT holds h/2)
    s2_np = (2.0 * g_wn / np.linalg.norm(v_wn, axis=1)).reshape(2, 1)
    s2_np = np.ascontiguousarray(s2_np, np.float32)
    bc_np = np.ascontiguousarray(b_cls.reshape(2, 1), np.float32)
    v_pad = np.zeros((384, 2), np.float32)
    v_pad[:H] = v_wn.T
    vt_np = np.ascontiguousarray(
        v_pad.reshape(3, 128, 2).transpose(1, 0, 2)).astype(
            ml_dtypes.bfloat16)

    emb_bf = embed.astype(ml_dtypes.bfloat16)

    # ---- per-core token streams and x layouts
    n0A, n0B = nA[0], nB[0]
    n0 = n0A + n0B
    in_maps = []
    for c in range(NCORES):
        toks = np.zeros(NTOKP, np.int64)
        for g in (0, 1):
            order = np.asarray(orders[c][g], np.int64)
            for t in range(T):
                n = nGs[g][t]
                if n == 0:
                    continue
                sel = order[:n]
                tk = np.where(sel >= 0, cap[np.clip(sel, 0, None), t], 0)
                toks[base[g] + offs[g][t]:base[g] + offs[g][t] + n] = tk
        X = emb_bf[toks]                      # [NTOKP, 300]
        XT = np.ascontiguousarray(X.T)        # [300, NTOKP]
        xab = np.zeros((128, 2, NTOKP), ml_dtypes.bfloat16)
        xab[:, 0, :] = XT[84:212]
        xab[0:88, 1, :] = XT[212:300]
        xab[88, 1, :] = 1.0
        x2 = np.ascontiguousarray(XT[0:84])   # -> xh[44:128, 2, :]
        # merged step-0 block: kt2 (zeros in h part) + kt3 + kt4
        xt0 = np.zeros((128, 3, n0), ml_dtypes.bfloat16)
        t0pos = np.concatenate(
            [np.arange(n0A), NA + np.arange(n0B)])
        xt0[44:128, 0, :] = XT[0:84][:, t0pos]
        xt0[:, 1, :] = XT[84:212][:, t0pos]
        xt0[0:88, 2, :] = XT[212:300][:, t0pos]
        xt0[88, 2, :] = 1.0
        in_maps.append({
            "wxh": wxh_np, "xab": xab, "x2": x2, "xt0": xt0,
            "vt": vt_np, "s2": s2_np, "bc": bc_np,
        })

    # ---- DMA issue plan: scalar engine gets only w4+xt0B (t0-critical,
    # issued before its first activation); everything else streams on the
    # sync queue in need-order.  xab/x2 cover [offs[1], N) only — xt0
    # carries the step-0 block.
    plan = [("sc", "w", 4),
            ("sc", "xt0", (NA, NA + n0B, n0A, n0)),
            ("sy", "w", 3),
            ("sy", "xt0", (0, n0A, 0, n0A)),
            ("sy", "w", 2), ("sy", "w", 0), ("sy", "w", 1)]
    cA = [int(offsA[t]) for t in (1, 2, 4, 8)] + [NA]
    cB = [NA + int(offsB[t]) for t in (1, 2, 4, 8)] + [NA + NB]
    plan += [("sy", "xab", (cA[0], cA[1])), ("sy", "xab", (cB[0], cB[1])),
             ("sy", "x2", (cA[0], cA[2])), ("sy", "x2", (cB[0], cB[2]))]
    for i in (1, 2, 3):
        plan += [("sy", "xab", (cA[i], cA[i + 1])),
                 ("sy", "xab", (cB[i], cB[i + 1]))]
        if i >= 2:
            plan += [("sy", "x2", (cA[i], cA[i + 1])),
                     ("sy", "x2", (cB[i], cB[i + 1]))]
    plan.append(("sy", "small", None))

    nc = _build_program(nGs, offs, base, NTOKP, CQ, n0, plan)
    return nc, in_maps, (orders, QA, QB)


def _unpack(results, meta):
    orders, QA, QB = meta
    out = np.zeros((B, 2), np.float32)
    for c in range(NCORES):
        logitsT = results[c]["out"]  # [2, CQ]
        for g, b0, Q in ((0, 0, QA), (1, QA, QB)):
            order = orders[c][g]
            for pos in range(Q):
                gi = order[pos]
                if gi >= 0:
                    out[gi] = logitsT[:, b0 + pos]
    return out


def _prep_and_run(inputs, trace=False):
    nc, in_maps, meta = _prepare(inputs)
    res = run_bass_kernel_spmd(nc, in_maps, list(range(NCORES)), trace=trace)
    return _unpack(res.results, meta), res


def kernel(**inputs):
    out, _ = _prep_and_run(inputs, trace=False)
    return out


# revision 30
# speedup vs baseline: 1.0710x; 1.0012x over previous
"""LSTM sequence classifier on 8 Trainium2 NeuronCores.

Data-parallel over batch: each core gets ~1/8 of the 4096 sequences.
Host pre-gathers token embeddings into a dense per-core stream (the
gather is pure data movement, done in numpy), so the device runs only
dense DMA + compute.  Per step the rhs operand packs [h; x; 1] into 5
K-tiles of 128 (h first so its partition layout matches the gate
layout; biases ride a constant-1 row), giving 12x5 matmuls per step.
All nonlinearities use sigmoid only (tanh x = 2*sigmoid(2x) - 1, with
the 2x folded into weights and the -0.5/x2 fixups folded into fused
DVE scalar_tensor_tensor ops; h is stored as h/2 with W_hh and the
head scale pre-doubled).  Gates live gate-major in PSUM: 4 banks per
step-segment, drained by a single sigmoid activation op.  Columns are
split into 2 interleaved groups x <=170-wide segments so ACT/DVE of
one unit overlap PE of the next and the recurrence never stalls PE.
"""
import sys

sys.path.insert(0, "/opt/trn_rl_repo")

import numpy as np
import ml_dtypes

import concourse.bass as bass
import concourse.tile as tile
from concourse import bacc, mybir
from concourse.bass_utils import run_bass_kernel_spmd

V, E, H, T, B = 30000, 300, 300, 22, 4096
NCORES = 8
KT = 5            # K-tiles: [h0, h1, h|x, x, x|1]
MW = 1536         # 4 gates x 384 padded rows
NMT = 12          # M-tiles
MAXW = 170        # max segment width (3*170 <= 512 psum bank f32)
F32 = mybir.dt.float32
BF16 = mybir.dt.bfloat16
FP16 = mybir.dt.float16
AF = mybir.ActivationFunctionType
ALU = mybir.AluOpType

_patched = False


def _patch_tile_drain():
    """walrus CTRL (Drain) supports fewer sem waits than Tile attaches at
    the kernel tail; spread them across single-wait SP NOPs instead."""
    global _patched
    if _patched:
        return
    _patched = True
    import concourse.tile as tile_mod
    from concourse.vector_clock import ScopedClock

    def _drain_and_barrier(self, tick_clock, wait_clock):
        nc = self.nc
        probe = nc.sync.nop(nofuse=True)
        wait_clock.add_sem_waits(
            probe.ins, ScopedClock({None: tick_clock.global_clock}))
        si = probe.ins.sync_info
        waits = list(si.on_wait) if si is not None else []
        upds = list(si.on_update) if si is not None else []
        probe.ins.sync_info = mybir.SyncInfo(on_wait=waits[:1], on_update=upds)
        for w in waits[1:]:
            n2 = nc.sync.nop(nofuse=True)
            n2.ins.sync_info = mybir.SyncInfo(on_wait=[w], on_update=[])
        nc.sync.drain()
        nc.all_engine_barrier()
        popped = nc._tile_sem_poison_stack.pop()
        assert popped is self._sem_poison
        nc.clear_and_free_semaphores(list(self.sems.allocated().values()))
        nc.all_engine_barrier()

    tile_mod.TileContext._drain_and_barrier = _drain_and_barrier


def _schedule(cap_len):
    """Deal batches to cores (identical length multiset per core), then
    deal each core's slots into 2 interleaved groups.

    Returns per-core per-group orders (global index or -1 for dummy) and
    per-group per-step active counts nA/nB (identical across cores).
    """
    orders = [([], []) for _ in range(NCORES)]
    qA = np.zeros(T + 1, np.int64)
    qB = np.zeros(T + 1, np.int64)
    toggle = 0
    for l in range(T, 0, -1):
        idxs = np.nonzero(cap_len == l)[0]
        ql = -(-len(idxs) // NCORES) if len(idxs) else 0
        parts = []
        for c in range(NCORES):
            p = [int(x) for x in idxs[c::NCORES]]
            parts.append(p + [-1] * (ql - len(p)))
        for j in range(ql):
            g = (toggle + j) % 2
            (qA if g == 0 else qB)[l] += 1
            for c in range(NCORES):
                orders[c][g].append(parts[c][j])
        toggle = (toggle + ql) % 2
    nA = [int(qA[t + 1:].sum()) for t in range(T)] + [0]
    nB = [int(qB[t + 1:].sum()) for t in range(T)] + [0]
    return orders, nA, nB


def _segments(n):
    """Split n active columns into balanced segments of width <= MAXW."""
    if n <= 0:
        return []
    S = -(-n // MAXW)
    w = -(-n // S)
    return [(s * w, min(n, (s + 1) * w)) for s in range(S)]


def _build_program(nG, offs, base, NTOKP, CQ, n0, dma_plan):
    nc = bacc.Bacc("TRN2", target_bir_lowering=False, debug=False)
    wxh_d = nc.dram_tensor("wxh", [KT, 128, MW], BF16, kind="ExternalInput")
    xab_d = nc.dram_tensor("xab", [128, 2, NTOKP], BF16, kind="ExternalInput")
    x2_d = nc.dram_tensor("x2", [84, NTOKP], BF16, kind="ExternalInput")
    xt0_d = nc.dram_tensor("xt0", [128, 3, n0], BF16, kind="ExternalInput")
    vt_d = nc.dram_tensor("vt", [128, 3, 2], BF16, kind="ExternalInput")
    s2_d = nc.dram_tensor("s2", [2, 1], F32, kind="ExternalInput")
    bc_d = nc.dram_tensor("bc", [2, 1], F32, kind="ExternalInput")
    out_d = nc.dram_tensor("out", [2, CQ], F32, kind="ExternalOutput")

    QA, QB = nG[0][0], nG[1][0]
    cbase = (0, QA)  # column base into cT/lastT/out_sb per group

    with tile.TileContext(nc) as tc:
        with (
            tc.tile_pool(name="const", bufs=1) as cpool,
            tc.tile_pool(name="gates", bufs=5) as gpool,
            tc.tile_pool(name="tsig", bufs=5) as tpool,
            tc.tile_pool(name="ps", bufs=4, space="PSUM") as pspool,
        ):
            wxh = cpool.tile([128, KT, MW], BF16, tag="wxh")
            xh = cpool.tile([128, KT, NTOKP], BF16, tag="xh")
            cT = cpool.tile([128, 3, CQ], FP16, tag="cT")
            lastT = cpool.tile([128, 3, CQ], BF16, tag="lastT")
            vt = cpool.tile([128, 3, 2], BF16, tag="vt")
            s2 = cpool.tile([2, 1], F32, tag="s2")
            bc = cpool.tile([2, 1], F32, tag="bc")
            out_sb = cpool.tile([2, CQ], F32, tag="out_sb")
            dum = cpool.tile([2, 2], F32, tag="dum")

            # DMA issue plan with explicit queues.  The scalar engine gets
            # ONLY the two t0-critical pieces: each dma_start occupies the
            # issuing engine until a HWDGE queue slot frees, so bulk DMAs
            # on the scalar engine would block the gate drains for ~20us.
            qeng = {"sy": nc.sync, "sc": nc.scalar}
            dummy_done = False
            for qi, (q, kind, a) in enumerate(dma_plan):
                eng = qeng[q]
                if q == "sy" and qi >= 2 and not dummy_done:
                    # preload the sigmoid table once the scalar engine has
                    # kicked its two critical DMAs
                    dummy_done = True
                    nc.vector.memset(dum[:], 0.0)
                    nc.scalar.activation(dum[:], dum[:], AF.Sigmoid)
                if kind == "w":
                    eng.dma_start(out=wxh[:, a, :], in_=wxh_d[a])
                elif kind == "wh":
                    k, m0, m1 = a
                    eng.dma_start(out=wxh[:, k, m0:m1], in_=wxh_d[k, :, m0:m1])
                elif kind == "xt0":
                    d0, d1, s0, s1 = a
                    if d1 > d0:
                        eng.dma_start(out=xh[:, 2:5, d0:d1],
                                      in_=xt0_d[:, :, s0:s1])
                elif kind == "x2":
                    p0, p1 = a
                    if p1 > p0:
                        eng.dma_start(out=xh[44:128, 2, p0:p1],
                                      in_=x2_d[:, p0:p1])
                elif kind == "xab":
                    p0, p1 = a
                    if p1 > p0:
                        eng.dma_start(out=xh[:, 3:5, p0:p1],
                                      in_=xab_d[:, :, p0:p1])
                elif kind == "small":
                    eng.dma_start(out=vt[:], in_=vt_d[:])
                    eng.dma_start(out=s2[:], in_=s2_d[:])
                    eng.dma_start(out=bc[:], in_=bc_d[:])

            for t in range(T):
                units = []
                for g in (0, 1):
                    for si, seg in enumerate(_segments(nG[g][t])):
                        units.append((si, g, seg))
                units.sort()
                for (si, g, (s0, s1)) in units:
                    w = s1 - s0
                    P = base[g] + offs[g][t] + s0
                    # psum: bank-order [g,i,f,o].  start=True zeroes a whole
                    # 2KB bank (zero region): one start per bank on its first
                    # write, one stop on its last.  When 6w<=512 a unit fits
                    # TWO gates per bank -> one 2-bank tile, so 4 units can
                    # be in flight (deeper pipeline for the short-step tail).
                    single = 6 * w <= 512
                    if single:
                        pt = pspool.tile([128, 2, 512], F32, tag="ps")
                        ptiles = (pt, pt)

                        def oslice(b, sub, w=w, pt=pt):
                            c0 = (b % 2) * 3 * w + sub * w
                            return pt[:, b // 2, c0:c0 + w]
                        bfirst = (True, False, True, False)
                        blast = (False, True, False, True)
                    else:
                        pa = pspool.tile([128, 2, 512], F32, tag="ps")
                        pb = pspool.tile([128, 2, 512], F32, tag="ps")
                        ptiles = (pa, pb)

                        def oslice(b, sub, w=w, pa=pa, pb=pb):
                            tile_ = pa if b < 2 else pb
                            return tile_[:, b % 2, sub * w:(sub + 1) * w]
                        bfirst = (True, True, True, True)
                        blast = (True, True, True, True)
                    gb = gpool.tile([128, 4, 3 * MAXW], FP16, tag="gb")
                    tg = tpool.tile([128, 3 * MAXW], FP16, tag="tg")
                    klist = [3, 4, 2] if t == 0 else [3, 4, 2, 0, 1]
                    # phase 1: x-only K-tiles (no dependence on h).
                    # k-major order: phase 2's k2 needs only the h-residue
                    # DVE write, so later h writes get time to land.
                    for k in klist[:2]:
                        for m in range(NMT):
                            b, sub = m // 3, m % 3
                            nc.tensor.matmul(
                                oslice(b, sub),
                                wxh[:, k, m * 128:(m + 1) * 128],
                                xh[:, k, P:P + w],
                                start=(sub == 0 and k == klist[0]
                                       and bfirst[b]),
                                stop=False)
                    # phase 2: K-tiles that need h
                    for k in klist[2:]:
                        for m in range(NMT):
                            b, sub = m // 3, m % 3
                            nc.tensor.matmul(
                                oslice(b, sub),
                                wxh[:, k, m * 128:(m + 1) * 128],
                                xh[:, k, P:P + w],
                                start=False,
                                stop=(sub == 2 and k == klist[-1]
                                      and blast[b]))
                    # split drain: banks [g,i] first so the DVE tmp op can
                    # start while ACT drains [f,o] — shortens the chain and
                    # the ACT blocking quantum
                    if single:
                        d1_in = ptiles[0][:, 0, 0:6 * w]
                        d2_in = ptiles[1][:, 1, 0:6 * w]
                    else:
                        d1_in = ptiles[0][:, 0:2, 0:3 * w]
                        d2_in = ptiles[1][:, 0:2, 0:3 * w]
                    nc.scalar.activation(
                        gb[:, 0:2, 0:3 * w], d1_in, AF.Sigmoid)
                    csl = cT[:, :, cbase[g] + s0:cbase[g] + s1]
                    # tmp = (sig_g - 0.5)*sig_i = i*tanh(g)/2  -> gate-i slot
                    nc.vector.scalar_tensor_tensor(
                        gb[:, 1, 0:3 * w], gb[:, 0, 0:3 * w], -0.5,
                        gb[:, 1, 0:3 * w], op0=ALU.add, op1=ALU.mult)
                    nc.scalar.activation(
                        gb[:, 2:4, 0:3 * w], d2_in, AF.Sigmoid)
                    if t == 0:
                        nc.vector.tensor_scalar(
                            csl, gb[:, 1, 0:3 * w], 2.0, None, op0=ALU.mult)
                    else:
                        # f*c -> gate-f slot ; c = tmp*2 + f*c
                        nc.vector.scalar_tensor_tensor(
                            gb[:, 2, 0:3 * w], gb[:, 2, 0:3 * w], 0.0,
                            csl, op0=ALU.add, op1=ALU.mult)
                        nc.vector.scalar_tensor_tensor(
                            csl, gb[:, 1, 0:3 * w], 2.0,
                            gb[:, 2, 0:3 * w], op0=ALU.mult, op1=ALU.add)
                    # tg = sigmoid(2c);  h/2 = (tg - 0.5) * sig_o
                    nc.scalar.activation(
                        tg[:, 0:3 * w], csl, AF.Sigmoid, scale=2.0)
                    ncol = nG[g][t + 1]
                    se = min(s1, ncol)  # survivors in [s0, se)
                    if se > s0:
                        Pn = base[g] + offs[g][t + 1] + s0
                        wl = se - s0
                        # k2's h-residue first: phase 2 starts on it
                        nc.vector.scalar_tensor_tensor(
                            xh[0:44, 2, Pn:Pn + wl],
                            tg[0:44, 2 * w:2 * w + wl], -0.5,
                            gb[0:44, 3, 2 * w:2 * w + wl],
                            op0=ALU.add, op1=ALU.mult)
                        for sub in (0, 1):
                            nc.vector.scalar_tensor_tensor(
                                xh[:, sub, Pn:Pn + wl],
                                tg[:, sub * w:sub * w + wl], -0.5,
                                gb[:, 3, sub * w:sub * w + wl],
                                op0=ALU.add, op1=ALU.mult)
                    sd = max(s0, ncol)  # dying in [sd, s1)
                    if s1 > sd:
                        r0, r1 = sd - s0, s1 - s0
                        for sub in range(3):
                            nc.vector.scalar_tensor_tensor(
                                lastT[:, sub, cbase[g] + sd:cbase[g] + s1],
                                tg[:, sub * w + r0:sub * w + r1], -0.5,
                                gb[:, 3, sub * w + r0:sub * w + r1],
                                op0=ALU.add, op1=ALU.mult)

            # head: logits^T = s2 * (v @ last^T) + bc ; lastT holds h/2
            for g, Q in ((0, QA), (1, QB)):
                pht = pspool.tile([128, 2, 512], F32, tag="ps")
                ph = pht[0:2, 0, :]
                for k in range(3):
                    nc.tensor.matmul(ph[:, 0:Q], vt[:, k, :],
                                     lastT[:, k, cbase[g]:cbase[g] + Q],
                                     start=(k == 0), stop=(k == 2))
                nc.vector.tensor_scalar(
                    out_sb[:, cbase[g]:cbase[g] + Q], ph[:, 0:Q],
                    s2[:], bc[:], op0=ALU.mult, op1=ALU.add)
            nc.sync.dma_start(out=out_d[:], in_=out_sb[:])

    nc.compile()
    return nc


def _prepare(inputs):
    """Build the program and per-core input maps. Returns
    (nc, in_maps, meta) where meta has what output-unpacking needs."""
    _patch_tile_drain()
    cap = np.asarray(inputs["cap"]).astype(np.int64)
    cap_len = np.asarray(inputs["cap_len"]).astype(np.int64)
    embed = np.asarray(inputs["embed"], np.float32)
    W_ih = np.asarray(inputs["W_ih"], np.float32)
    W_hh = np.asarray(inputs["W_hh"], np.float32)
    b_ih = np.asarray(inputs["b_ih"], np.float32)
    b_hh = np.asarray(inputs["b_hh"], np.float32)
    v_wn = np.asarray(inputs["v_wn"], np.float32)
    g_wn = np.asarray(inputs["g_wn"], np.float32)
    b_cls = np.asarray(inputs["b_cls"], np.float32)

    orders, nA, nB = _schedule(cap_len)
    nGs = (nA, nB)
    offsA = np.concatenate([[0], np.cumsum(nA[:T])]).astype(np.int64)
    offsB = np.concatenate([[0], np.cumsum(nB[:T])]).astype(np.int64)
    NA, NB = int(offsA[T]), int(offsB[T])
    QA, QB = nA[0], nB[0]
    CQ = QA + QB
    NTOK = NA + NB
    NTOKP = NTOK + (-NTOK) % 16
    base = (0, NA)
    offs = (offsA, offsB)

    # ---- weights: contract rows [h(300)*2 ; x(300) ; 1-bias], M = 4x384
    # bank order g,i,f,o ; gate g rows are doubled for tanh-as-sigmoid.
    Wk = np.zeros((KT * 128, MW), np.float32)
    bias = b_ih + b_hh
    for b, gidx in enumerate((2, 0, 1, 3)):
        rows = slice(H * gidx, H * gidx + H)
        scale = 2.0 if gidx == 2 else 1.0
        Wk[0:H, 384 * b:384 * b + H] = 2.0 * scale * W_hh[rows, :].T
        Wk[300:600, 384 * b:384 * b + H] = scale * W_ih[rows, :].T
        Wk[600, 384 * b:384 * b + H] = scale * bias[rows]
    wxh_np = np.ascontiguousarray(
        Wk.reshape(KT, 128, MW)).astype(ml_dtypes.bfloat16)

    # head: s = 2 * g / ||v|| (factor 2 since lastT holds h/2)
    s2_np = (2.0 * g_wn / np.linalg.norm(v_wn, axis=1)).reshape(2, 1)
    s2_np = np.ascontiguousarray(s2_np, np.float32)
    bc_np = np.ascontiguousarray(b_cls.reshape(2, 1), np.float32)
    v_pad = np.zeros((384, 2), np.float32)
    v_pad[:H] = v_wn.T
    vt_np = np.ascontiguousarray(
        v_pad.reshape(3, 128, 2).transpose(1, 0, 2)).astype(
            ml_dtypes.bfloat16)

    emb_bf = embed.astype(ml_dtypes.bfloat16)

    # ---- per-core token streams and x layouts
    n0A, n0B = nA[0], nB[0]
    n0 = n0A + n0B
    in_maps = []
    for c in range(NCORES):
        toks = np.zeros(NTOKP, np.int64)
        for g in (0, 1):
            order = np.asarray(orders[c][g], np.int64)
            for t in range(T):
                n = nGs[g][t]
                if n == 0:
                    continue
                sel = order[:n]
                tk = np.where(sel >= 0, cap[np.clip(sel, 0, None), t], 0)
                toks[base[g] + offs[g][t]:base[g] + offs[g][t] + n] = tk
        X = emb_bf[toks]                      # [NTOKP, 300]
        XT = np.ascontiguousarray(X.T)        # [300, NTOKP]
        xab = np.zeros((128, 2, NTOKP), ml_dtypes.bfloat16)
        xab[:, 0, :] = XT[84:212]
        xab[0:88, 1, :] = XT[212:300]
        xab[88, 1, :] = 1.0
        x2 = np.ascontiguousarray(XT[0:84])   # -> xh[44:128, 2, :]
        # merged step-0 block: kt2 (zeros in h part) + kt3 + kt4
        xt0 = np.zeros((128, 3, n0), ml_dtypes.bfloat16)
        t0pos = np.concatenate(
            [np.arange(n0A), NA + np.arange(n0B)])
        xt0[44:128, 0, :] = XT[0:84][:, t0pos]
        xt0[:, 1, :] = XT[84:212][:, t0pos]
        xt0[0:88, 2, :] = XT[212:300][:, t0pos]
        xt0[88, 2, :] = 1.0
        in_maps.append({
            "wxh": wxh_np, "xab": xab, "x2": x2, "xt0": xt0,
            "vt": vt_np, "s2": s2_np, "bc": bc_np,
        })

    # ---- DMA issue plan: scalar engine gets only w4+xt0B (t0-critical,
    # issued before its first activation); everything else streams on the
    # sync queue in need-order.  xab/x2 cover [offs[1], N) only — xt0
    # carries the step-0 block.
    h0A = min(n0A, 170)
    h0B = min(n0B, 170)
    plan = [("sc", "wh", (4, 0, 768)),
            ("sc", "xt0", (NA, NA + h0B, n0A, n0A + h0B)),
            ("sc", "wh", (4, 768, MW)),
            ("sc", "xt0", (NA + h0B, NA + n0B, n0A + h0B, n0)),
            ("sy", "wh", (3, 0, 768)),
            ("sy", "xt0", (0, h0A, 0, h0A)),
            ("sy", "wh", (3, 768, MW)),
            ("sy", "xt0", (h0A, n0A, h0A, n0A)),
            ("sy", "w", 2), ("sy", "w", 0), ("sy", "w", 1)]
    cA = [int(offsA[t]) for t in (1, 2, 4, 8)] + [NA]
    cB = [NA + int(offsB[t]) for t in (1, 2, 4, 8)] + [NA + NB]
    plan += [("sy", "xab", (cA[0], cA[1])), ("sy", "xab", (cB[0], cB[1])),
             ("sy", "x2", (cA[0], cA[2])), ("sy", "x2", (cB[0], cB[2]))]
    for i in (1, 2, 3):
        plan += [("sy", "xab", (cA[i], cA[i + 1])),
                 ("sy", "xab", (cB[i], cB[i + 1]))]
        if i >= 2:
            plan += [("sy", "x2", (cA[i], cA[i + 1])),
                     ("sy", "x2", (cB[i], cB[i + 1]))]
    plan.append(("sy", "small", None))

    nc = _build_program(nGs, offs, base, NTOKP, CQ, n0, plan)
    return nc, in_maps, (orders, QA, QB)


def _unpack(results, meta):
    orders, QA, QB = meta
    out = np.zeros((B, 2), np.float32)
    for c in range(NCORES):
        logitsT = results[c]["out"]  # [2, CQ]
        for g, b0, Q in ((0, 0, QA), (1, QA, QB)):
            order = orders[c][g]
            for pos in range(Q):
                gi = order[pos]
                if gi >= 0:
                    out[gi] = logitsT[:, b0 + pos]
    return out


def _prep_and_run(inputs, trace=False):
    nc, in_maps, meta = _prepare(inputs)
    res = run_bass_kernel_spmd(nc, in_maps, list(range(NCORES)), trace=trace)
    return _unpack(res.results, meta), res


def kernel(**inputs):
    out, _ = _prep_and_run(inputs, trace=False)
    return out
